# revision 1
# baseline (speedup 1.0000x reference)
"""Trainium2 Bass kernel for nn_Conv1dAttention.

Math (per sample):
  q,k,v,pe = lrelu(bn(conv1d(x, W_p)))           # [C=128, L=2048], Cin=64, K=3
  S = q^T k                                      # [L, L]
  P = softmax_rows(S)                            # softmax over last axis
  out = v @ P + pe                               # [C, L]

Sharding: data-parallel over batch B=16 across 8 NeuronCores (2 samples/core).
Same NEFF on all cores, per-core input shards, no collectives.

Design notes (v2 — exp-stream centric):
  - ScalarE exp is the hard floor: 64 ACTIVATEs x (1024+352)/1.2ns ~= 73us.
    Everything else is scheduled to hide under that stream.
  - Single x load per sample: xs [128, L+1] with rows 0-63 = x shifted right
    (k=0 tap) and rows 64-127 = x direct (k=1 tap). The k=2 tap reads rows
    64-127 through a +1 column offset AP, so no third copy and no xs2 DMA.
  - BN folded into weights on host. Conv bias applied in the lrelu drain as a
    per-partition tensor_scalar add (free: the drain pass is mandatory anyway).
    V^T's bias (free-axis) is injected with a K=1 ones-row matmul.
  - lrelu = max(y+b, 0.3(y+b)) in two DVE ops: TS-add psum->bf16, STT max.
  - Softmax Z on DVE for BOTH samples (STT 2x-mode add with accum_out);
    ScalarE does nothing but exp.
  - Output in bf16 (upcast on host): halves the out DMA; DVE drain adds pe.
  - DMA issue is the prelude killer (~650ns per dma_start on the issuing
    queue): weights packed into 4 tensors, x is 2 dma_starts per sample,
    split across both HWDGE queues (sync + scalar).
  - Phase B = s0 attention with s0-leftover + s1 q/k conv filler; phase C =
    s1 attention with s1 v/pe filler (balances PE per phase). PE warmup mms
    keep the HAM clock-gate at 2.4 GHz through the DMA wait.
"""

import sys

if "/opt/trn_rl_repo" not in sys.path:
    sys.path.insert(0, "/opt/trn_rl_repo")

from contextlib import ExitStack

import ml_dtypes
import numpy as np

import concourse.bass as bass
import concourse.tile as tile
from concourse import bacc, mybir
from concourse.bass_utils import run_bass_kernel_spmd

B, CIN, COUT, KW, L = 16, 64, 128, 3, 2048
NCORES = 8
BP = B // NCORES  # samples per core
EPS = 1e-5
SLOPE = 0.3
F32 = mybir.dt.float32
BF16 = mybir.dt.bfloat16
NB = L // 128  # 16 a-blocks
HALF = 1024

_CACHE = {}

PCOL = {"q": 0, "k": 1, "v": 2, "p": 3}


def _body(ctx, tc, x, w12, w3, bc, bv, out):
    nc = tc.nc
    amax = mybir.AluOpType.max
    mult = mybir.AluOpType.mult
    Exp = mybir.ActivationFunctionType.Exp

    wpool = ctx.enter_context(tc.tile_pool(name="wpool", bufs=1))
    xpool = ctx.enter_context(tc.tile_pool(name="xpool", bufs=2))
    apool = ctx.enter_context(tc.tile_pool(name="apool", bufs=2))
    ppool = ctx.enter_context(tc.tile_pool(name="ppool", bufs=5))
    opool = ctx.enter_context(tc.tile_pool(name="opool", bufs=2))
    vpool = ctx.enter_context(tc.tile_pool(name="vpool", bufs=5))
    zpool = ctx.enter_context(tc.tile_pool(name="zpool", bufs=4))
    lpool = ctx.enter_context(tc.tile_pool(name="lpool", bufs=2))
    psA = ctx.enter_context(tc.tile_pool(name="psA", bufs=2, space="PSUM"))
    psO = ctx.enter_context(tc.tile_pool(name="psO", bufs=1, space="PSUM"))

    # --- weight / param tiles (few, packed DMAs; scalar queue is idle early)
    w12_t = wpool.tile([128, 4 * COUT], BF16, tag="w12", name="w12")
    w3_t = wpool.tile([CIN, 4 * COUT], BF16, tag="w3", name="w3")
    bc_t = wpool.tile([128, 4], F32, tag="bc", name="bc")
    bv_t = wpool.tile([1, COUT], BF16, tag="bv", name="bv")
    ones1 = wpool.tile([1, COUT], BF16, tag="ones1", name="ones1")
    nc.gpsimd.memset(ones1[:, :], 1.0)
    nc.scalar.dma_start(out=w12_t[:, :], in_=w12[:, :])
    nc.sync.dma_start(out=bc_t[:, :], in_=bc[:, :])
    nc.scalar.dma_start(out=w3_t[:, :], in_=w3[:, :])
    nc.sync.dma_start(out=bv_t[:, :], in_=bv[:, :])

    def w12s(p):
        return w12_t[:, PCOL[p] * COUT : (PCOL[p] + 1) * COUT]

    def w3s(p):
        return w3_t[:, PCOL[p] * COUT : (PCOL[p] + 1) * COUT]

    def emit_xs(s, split=False):
        # xs rows 0-63: cols 0..L-1 = x, col L = 0   (x[cin, l] / +1 view)
        # xs rows 64-127: col 0 = 0, cols 1..L = x   (x[cin, l-1] at col l)
        xs = xpool.tile([128, L + 1], BF16, tag="xs", name="xs")
        nc.gpsimd.memset(xs[0:CIN, L : L + 1], 0.0)
        nc.gpsimd.memset(xs[CIN:128, 0:1], 0.0)
        if split:
            nc.scalar.dma_start(out=xs[0:CIN, 0:HALF], in_=x[s, :, 0:HALF])
            nc.sync.dma_start(out=xs[CIN:128, 1 : HALF + 1], in_=x[s, :, 0:HALF])
            nc.scalar.dma_start(out=xs[0:CIN, HALF:L], in_=x[s, :, HALF:L])
            nc.sync.dma_start(out=xs[CIN:128, HALF + 1 : L + 1], in_=x[s, :, HALF:L])
        else:
            nc.sync.dma_start(out=xs[0:CIN, 0:L], in_=x[s, :, :])
            nc.sync.dma_start(out=xs[CIN:128, 1 : L + 1], in_=x[s, :, :])
        return xs

    Prelu = mybir.ActivationFunctionType.Prelu

    def drain(dst_ap, ps_ap, bias_ap, eng="v"):
        # lrelu(y + b) = max(z, 0.3z), z = y + b.
        # eng="s": one ScalarE Lrelu op (used when the exp stream is idle or
        # has slack). eng="v": two DVE passes (TS bias-add psum->bf16, STT max).
        if eng == "s":
            nc.scalar.activation(
                dst_ap,
                ps_ap,
                Prelu,
                bias=bias_ap if bias_ap is not None else 0.0,
                scale=1.0,
                alpha=SLOPE,
            )
            return
        w = ps_ap.free_size()
        yb = lpool.tile([128, w], BF16, tag="yb", name="yb")
        if bias_ap is None:
            nc.vector.tensor_scalar_mul(yb[:, :], ps_ap, 1.0)
        else:
            nc.vector.tensor_scalar_add(yb[:, :], ps_ap, bias_ap)
        nc.vector.scalar_tensor_tensor(
            dst_ap, yb[:, :], SLOPE, yb[:, :], op0=mult, op1=amax
        )

    def conv_q(xs, p, dst, q, eng="v"):
        # one [128,512] quarter of a [c, l]-layout conv
        cps = psA.tile([128, 512], F32, tag="ps", name="cps")
        c0 = q * 512
        nc.tensor.matmul(
            cps[:, :], w12s(p), xs[:, c0 : c0 + 512], start=True, stop=False
        )
        nc.tensor.matmul(
            cps[:, :],
            w3s(p),
            xs[0:CIN, c0 + 1 : c0 + 513],
            start=False,
            stop=True,
        )
        drain(dst[:, c0 : c0 + 512], cps[:, :], bc_t[:, PCOL[p] : PCOL[p] + 1], eng)

    def conv_h(xs, p, dst, h, eng="v"):
        # one [128,1024] half of a [c, l]-layout conv: 4 matmuls (2 per
        # 512-col psum bank), one drain pass-pair.
        cps = psA.tile([128, HALF], F32, tag="ps", name="cph")
        for qq in range(2):
            c0 = h * HALF + qq * 512
            pc = slice(qq * 512, qq * 512 + 512)
            nc.tensor.matmul(
                cps[:, pc], w12s(p), xs[:, c0 : c0 + 512], start=True, stop=False
            )
            nc.tensor.matmul(
                cps[:, pc],
                w3s(p),
                xs[0:CIN, c0 + 1 : c0 + 513],
                start=False,
                stop=True,
            )
        drain(
            dst[:, h * HALF : (h + 1) * HALF],
            cps[:, :],
            bc_t[:, PCOL[p] : PCOL[p] + 1],
            eng,
        )

    def vt_qgroup(xs, vt, gh, eng="s"):
        # 4 l-blocks of V in transposed [l, c] layout -> one [128,512] tile.
        # Stationary = xs slices (contraction over cin); bias via K=1 ones mm.
        vps = psA.tile([128, 512], F32, tag="ps", name="vps")
        for i in range(4):
            blk = gh * 4 + i
            c = blk * 128
            pc = slice(i * 128, i * 128 + 128)
            nc.tensor.matmul(
                vps[:, pc], xs[:, c : c + 128], w12s("v"), start=True, stop=False
            )
            nc.tensor.matmul(
                vps[:, pc],
                xs[0:CIN, c + 1 : c + 129],
                w3s("v"),
                start=False,
                stop=False,
            )
            nc.tensor.matmul(
                vps[:, pc], ones1[0:1, :], bv_t[0:1, :], start=False, stop=True
            )
        drain(vt[:, gh * 512 : (gh + 1) * 512], vps[:, :], None, eng)

    def make_tiles():
        q_t = apool.tile([128, L], BF16, tag="actq", name="actq")
        k_t = apool.tile([128, L], BF16, tag="actk", name="actk")
        pe_t = apool.tile([128, L], BF16, tag="actp", name="actp")
        vt = apool.tile([128, L], BF16, tag="vt", name="vt")
        return q_t, k_t, pe_t, vt

    def attn_part1(tiles, blk):
        # S matmuls + exp for one 128-row block.
        q_t, k_t, pe_t, vt = tiles
        pblk = ppool.tile([128, L], BF16, tag="pblk", name="pblk")
        for h in range(2):
            sps = psA.tile([128, HALF], F32, tag="ps", name="sps")
            for n in range(2):
                c0 = h * HALF + n * 512
                nc.tensor.matmul(
                    sps[:, n * 512 : n * 512 + 512],
                    q_t[:, blk * 128 : blk * 128 + 128],
                    k_t[:, c0 : c0 + 512],
                    start=True,
                    stop=True,
                )
            nc.scalar.activation(pblk[:, h * HALF : (h + 1) * HALF], sps[:, :], Exp)
        return pblk

    def attn_part2(tiles, blk, pblk):
        # Z (softmax row sum) via one DVE copy-with-accumulate over the bf16
        # P block (eligible for the packed perf modes), then 1/Z onto vts.
        vt = tiles[3]
        z = zpool.tile([128, 1], F32, tag="z", name="z")
        zscr = lpool.tile([128, HALF], BF16, tag="zscr", name="zscr")
        nc.vector.scalar_tensor_tensor(
            zscr[:, :],
            pblk[:, 0:HALF],
            1.0,
            pblk[:, HALF:L],
            op0=mult,
            op1=mybir.AluOpType.add,
            accum_out=z[:, :],
        )
        r = zpool.tile([128, 1], F32, tag="r", name="r")
        nc.vector.reciprocal(r[:, :], z[:, :])
        vts = vpool.tile([128, 128], BF16, tag="vts", name="vts")
        nc.vector.tensor_scalar_mul(
            vts[:, :], vt[:, blk * 128 : blk * 128 + 128], r[:, :]
        )
        return vts

    def out_mms(out_ps, pblk, vts, blk):
        for n in range(4):
            nc.tensor.matmul(
                out_ps[:, n * 512 : n * 512 + 512],
                vts[:, :],
                pblk[:, n * 512 : n * 512 + 512],
                start=(blk == 0),
                stop=(blk == NB - 1),
            )

    def finish_sample(tiles, out_ps, s, nchunks=2):
        pe_t = tiles[2]
        outs = opool.tile([128, L], BF16, tag="outs", name="outs")
        cw = L // nchunks
        for h in range(nchunks):
            cols = slice(h * cw, (h + 1) * cw)
            nc.vector.tensor_tensor(
                outs[:, cols], out_ps[:, cols], pe_t[:, cols], mybir.AluOpType.add
            )
            nc.sync.dma_start(out=out[s, :, cols], in_=outs[:, cols])

    def finish_chunk(tiles, out_ps, s, n):
        pe_t = tiles[2]
        outs = opool.tile([128, 512], BF16, tag=f"outc{n}", name=f"outc{n}")
        cols = slice(n * 512, (n + 1) * 512)
        nc.vector.tensor_tensor(
            outs[:, :], out_ps[:, cols], pe_t[:, cols], mybir.AluOpType.add
        )
        nc.sync.dma_start(out=out[s, :, cols], in_=outs[:, :])

    def attention_phase(tiles, out_ps, queue, finish=None):
        # Software-pipelined two blocks deep: block b's S/exp run ahead of
        # block b-2's out-matmuls, so the DVE z->vts round trip never gates
        # the PE. Filler drains are emitted before the z chain so their psA
        # slots free early. `queue` = (deadline_blk, thunk) conv filler units.
        qi = 0
        pend = []
        for blk in range(NB):
            while qi < len(queue) and queue[qi][0] <= blk:
                queue[qi][1]()
                qi += 1
            pblk = attn_part1(tiles, blk)
            while qi < len(queue) and (qi + 1) * (NB - 2) <= blk * len(queue):
                queue[qi][1]()
                qi += 1
            vts = attn_part2(tiles, blk, pblk)
            pend.append((pblk, vts, blk))
            if len(pend) > 3:
                out_mms(out_ps, *pend.pop(0))
        if finish is None:
            return pend
        # flush: all but the last block normally, then column-by-column with
        # the finish chunk for column n right after its final matmul.
        for p in pend[:-1]:
            out_mms(out_ps, *p)
        pblk, vts, blk = pend[-1]
        for n in range(4):
            nc.tensor.matmul(
                out_ps[:, n * 512 : n * 512 + 512],
                vts[:, :],
                pblk[:, n * 512 : n * 512 + 512],
                start=(blk == 0),
                stop=True,
            )
            finish(n)

    assert BP == 2
    # PE warm-up: dummy matmuls on a memset tile keep the PE busy through the
    # x-DMA wait so the HAM clock-gate reaches 2.4 GHz before the real work.
    wseed = wpool.tile([128, 128], BF16, tag="wseed", name="wseed")
    nc.gpsimd.memset(wseed[:, :], 0.001)
    wps = psA.tile([128, 128], F32, tag="ps", name="wps")
    for _ in range(40):
        nc.tensor.matmul(wps[:, :], wseed[:, :], wseed[:, :], start=True, stop=True)

    xs0 = emit_xs(0, split=True)
    xs1 = emit_xs(1)
    tiles0 = make_tiles()
    q0, k0, pe0, vt0 = tiles0
    tiles1 = make_tiles()
    q1, k1, pe1, vt1 = tiles1

    # Prelude: only what attention block 0 needs — Q quarter 0, all of K,
    # first V^T group. Drains on ScalarE (idle until the first exp).
    conv_q(xs0, "q", q0, 0, eng="s")
    conv_h(xs0, "k", k0, 0, eng="s")
    conv_h(xs0, "k", k0, 1, eng="s")
    vt_qgroup(xs0, vt0, 0, eng="s")
    # Sample-1 q/k convs also in the prelude: the psA ring has no exp
    # contention yet, and DVE idles during the x DMA wait. Keeping them out
    # of phase B removes the per-filler-block PE-issue bubble (filler mms
    # wait for the psA slot the in-flight exp is reading).
    conv_h(xs1, "k", k1, 0)
    conv_h(xs1, "k", k1, 1)
    conv_h(xs1, "q", q1, 0)
    conv_h(xs1, "q", q1, 1)

    queueB = [
        (4, lambda: conv_q(xs0, "q", q0, 1)),
        (4, lambda: vt_qgroup(xs0, vt0, 1)),
        (8, lambda: conv_q(xs0, "q", q0, 2)),
        (8, lambda: vt_qgroup(xs0, vt0, 2)),
        (12, lambda: conv_q(xs0, "q", q0, 3)),
        (12, lambda: vt_qgroup(xs0, vt0, 3)),
    ]
    queueB.append((99, lambda: conv_h(xs0, "p", pe0, 0, eng="s")))
    queueB.append((99, lambda: conv_h(xs0, "p", pe0, 1, eng="s")))

    queueC = [
        (0, lambda: vt_qgroup(xs1, vt1, 0)),
        (4, lambda: vt_qgroup(xs1, vt1, 1)),
        (8, lambda: vt_qgroup(xs1, vt1, 2)),
        (12, lambda: vt_qgroup(xs1, vt1, 3)),
    ]
    for q in range(4):
        queueC.append((99, lambda q=q: conv_q(xs1, "p", pe1, q)))

    out_ps0 = psO.tile([128, L], F32, tag="ops", name="out_ps0")
    pend0 = attention_phase(tiles0, out_ps0, queueB)
    # Stage sample-0's out-matmul flush + drain into sample-1's early blocks
    # so phase C's first S-matmuls are not queued behind it on the PE.
    queueC = [
        (0, lambda: vt_qgroup(xs1, vt1, 0, eng="s")),
        (0, lambda: out_mms(out_ps0, *pend0[0])),
        (1, lambda: out_mms(out_ps0, *pend0[1])),
        (2, lambda: out_mms(out_ps0, *pend0[2])),
        (2, lambda: finish_sample(tiles0, out_ps0, 0)),
    ] + queueC[1:]
    out_ps1 = psO.tile([128, L], F32, tag="ops", name="out_ps1")
    attention_phase(tiles1, out_ps1, queueC, finish=lambda n: finish_chunk(tiles1, out_ps1, 1, n))

    


def build():
    nc = bacc.Bacc("TRN2", target_bir_lowering=False, debug=False)
    x_d = nc.dram_tensor("x", [BP, CIN, L], BF16, kind="ExternalInput")
    w12_d = nc.dram_tensor("w12", [128, 4 * COUT], BF16, kind="ExternalInput")
    w3_d = nc.dram_tensor("w3", [CIN, 4 * COUT], BF16, kind="ExternalInput")
    bc_d = nc.dram_tensor("bc", [128, 4], F32, kind="ExternalInput")
    bv_d = nc.dram_tensor("bv", [1, COUT], BF16, kind="ExternalInput")
    out_d = nc.dram_tensor("out", [BP, COUT, L], BF16, kind="ExternalOutput")

    with tile.TileContext(nc) as tc, ExitStack() as ctx:
        _body(
            ctx,
            tc,
            x_d.ap(),
            w12_d.ap(),
            w3_d.ap(),
            bc_d.ap(),
            bv_d.ap(),
            out_d.ap(),
        )
    nc.compile()
    return nc


def _fold_weights(w, b, gamma, beta, mean, var):
    """Fold BN affine (fixed mean/var) into conv weights; split by tap."""
    w = np.asarray(w, np.float64)
    scale = np.asarray(gamma, np.float64) / np.sqrt(np.asarray(var, np.float64) + EPS)
    shift = np.asarray(beta, np.float64) - np.asarray(mean, np.float64) * scale
    wf = w * scale[:, None, None]  # [COUT, CIN, K]
    bf = np.asarray(b, np.float64) * scale + shift
    w12 = np.empty((128, COUT), np.float32)
    w12[0:CIN] = wf[:, :, 1].T
    w12[CIN:128] = wf[:, :, 0].T
    w3 = np.ascontiguousarray(wf[:, :, 2].T.astype(np.float32))  # [CIN, COUT]
    return w12, w3, bf.astype(np.float32)


def _get_nc():
    if "nc" not in _CACHE:
        _CACHE["nc"] = build()
    return _CACHE["nc"]


def make_in_maps(inputs):
    bf = ml_dtypes.bfloat16
    x = np.ascontiguousarray(np.asarray(inputs["x"], np.float32).astype(bf))
    folded = {}
    for p in "qkvp":
        key = p if p != "p" else "pe"
        folded[p] = _fold_weights(
            inputs[f"{key}_w"],
            inputs[f"{key}_b"],
            inputs[f"{key}_gamma"],
            inputs[f"{key}_beta"],
            inputs[f"{key}_mean"],
            inputs[f"{key}_var"],
        )
    w12pack = np.concatenate([folded[p][0] for p in "qkvp"], axis=1).astype(bf)
    w3pack = np.concatenate([folded[p][1] for p in "qkvp"], axis=1).astype(bf)
    bcols = np.stack([folded[p][2] for p in "qkvp"], axis=1).astype(np.float32)
    bvrow = np.ascontiguousarray(folded["v"][2][None, :]).astype(bf)
    in_maps = []
    for i in range(NCORES):
        m = {
            "x": np.ascontiguousarray(x[i * BP : (i + 1) * BP]),
            "w12": w12pack,
            "w3": w3pack,
            "bc": bcols,
            "bv": bvrow,
        }
        in_maps.append(m)
    return in_maps


def kernel(**inputs):
    nc = _get_nc()
    in_maps = make_in_maps(inputs)
    res = run_bass_kernel_spmd(nc, in_maps, core_ids=list(range(NCORES)))
    out = np.concatenate([res.results[i]["out"] for i in range(NCORES)], axis=0)
    return out.astype(np.float32)


if __name__ == "__main__":
    rng = np.random.default_rng(0)
    ins = {"x": rng.standard_normal((B, CIN, L), dtype=np.float32)}
    for p in ("q", "k", "v", "pe"):
        ins[f"{p}_w"] = (rng.standard_normal((COUT, CIN, KW)) * 0.05).astype(np.float32)
        ins[f"{p}_b"] = (rng.standard_normal(COUT) * 0.05).astype(np.float32)
        ins[f"{p}_gamma"] = rng.uniform(0.5, 1.5, COUT).astype(np.float32)
        ins[f"{p}_beta"] = (rng.standard_normal(COUT) * 0.05).astype(np.float32)
        ins[f"{p}_mean"] = (rng.standard_normal(COUT) * 0.05).astype(np.float32)
        ins[f"{p}_var"] = rng.uniform(0.5, 1.5, COUT).astype(np.float32)
    got = kernel(**ins)
    print("kernel output:", got.shape, got.dtype, np.abs(got).mean())

